# revision 6
# baseline (speedup 1.0000x reference)
"""Trainium2 Bass kernel for a 2-layer ConvGRU (L=512, T=96, C=H=150, K=5).

Sharding: spatial axis L split over 8 NeuronCores (64 owned positions each).
Each core computes a 128-wide region (owned + 32 halo per side). Halo
validity decays 2 positions/step (kernel-5 SAME conv); it is refreshed every
16 steps by a ReduceScatter halo exchange (per-core one-hot masks route each
core's boundary strips to its neighbours' slots, keeping the SPMD program
uniform). Layer-0 exchanges sit at t=16k, layer-1 at t=16k+8, so each
layer's collective latency hides under the other layer's compute.

TensorEngine work per GRU step and layer: 5 main-channel taps + a packed
4-tap channel-remainder block (32-aligned sub-blocks, zero gap rows) + a
tap-4/bias block, per gate group ([r|z] into one PSUM bank, [n] into
another), plus a 2-matmul identity pass that re-materialises h_{t-1}
position-major straight from the exchanged channel-major state. Gate math
runs on ACT (sigmoid/tanh) + DVE; h_new returns to channel-major via two PE
transposes.

All input reshaping (channel-major transposes, tap-shifted im2col packing,
bias/valid rows, routing masks) is done host-side in numpy inside kernel().
"""

import os
import sys
import types

import numpy as np

if "/opt/trn_rl_repo" not in sys.path:
    sys.path.insert(0, "/opt/trn_rl_repo")


def _install_ntff_hook():
    # antenv.axon_hooks is absent from this image; recreate the registry and
    # register the ctypes NTFF hook so trace=True yields exec_time_ns.
    try:
        import antenv
        if "antenv.axon_hooks" in sys.modules:
            return True
        mod = types.ModuleType("antenv.axon_hooks")
        _hook = [None]
        mod.set_axon_ntff_profile_hook = lambda h: _hook.__setitem__(0, h)
        mod.get_axon_ntff_profile_hook = lambda: _hook[0]
        sys.modules["antenv.axon_hooks"] = mod
        antenv.axon_hooks = mod
        from trn_agent_boot.trn_boot import _ntff_profile_via_ctypes
        mod.set_axon_ntff_profile_hook(
            _ntff_profile_via_ctypes("/opt/axon/libaxon_pjrt.so"))
        return True
    except Exception:
        return False


L, C, H, KW = 512, 150, 150, 5
G3 = 3 * H            # 450
NCORES = 8
OWN = 64              # owned positions per core
SW = 134              # stored width per timestep (128 region + 2 pads/side)
W0 = 12               # rolling slots for layer-0 output trajectory
LAG = 4               # emission lag of layer 1 behind layer 0
CL = 22               # channel remainder (150 - 128)
AR = 118              # 4 tap blocks at partitions {0,32,64,96}, 22 rows each
BR = 23               # tap-4 rows + ones/bias row

# tap-4 channel -> spare row inside the 118-row aligned block (layer-0 xs
# packing only): gaps 22..31, 54..63, 86..87; ones/bias row at 88.
_GAP_PACK = [(22 + c, c) for c in range(10)] + \
    [(54 + c - 10, c) for c in range(10, 20)] + \
    [(86 + c - 20, c) for c in range(20, 22)]
_GAP_PACK = [(r, c) for r, c in _GAP_PACK]
_ONES_ROW = 88

LAST_EXEC_NS = None


def build(T):
    import concourse.bacc as bacc
    import concourse.mybir as mybir
    import concourse.tile as tile

    f32 = mybir.dt.float32
    nc = bacc.Bacc("TRN2", target_bir_lowering=False, debug=False,
                   num_devices=NCORES)

    d_xsa = nc.dram_tensor("xsT_a", [128, T * SW], f32, kind="ExternalInput")
    d_xsba = nc.dram_tensor("xsT_bA", [AR, T * SW], f32, kind="ExternalInput")
    wnames = []
    for l in range(2):
        for nm in (f"wim{l}", f"whm{l}"):
            wnames.append((nm, [128, KW * G3]))
        wnames.append((f"wilA{l}", [AR, G3]))
        wnames.append((f"whlA{l}", [AR, G3]))
        wnames.append((f"whlB{l}", [CL, G3]))
    wnames.append(("wilB1", [BR, G3]))
    d_w = {nm: nc.dram_tensor(nm, sh, f32, kind="ExternalInput")
           for nm, sh in wnames}
    d_maska = nc.dram_tensor("mask_a", [128, 512], f32, kind="ExternalInput")
    d_maskb = nc.dram_tensor("mask_b", [CL, 512], f32, kind="ExternalInput")
    d_valid = nc.dram_tensor("valid128", [1, 128], f32, kind="ExternalInput")
    d_validp = nc.dram_tensor("validp", [128, 1], f32, kind="ExternalInput")
    d_ident = nc.dram_tensor("ident", [128, 128], f32, kind="ExternalInput")
    d_imain = nc.dram_tensor("imain", [128, H], f32, kind="ExternalInput")
    d_ileft = nc.dram_tensor("ileft", [CL, H], f32, kind="ExternalInput")
    d_out = nc.dram_tensor("out", [T, OWN, H], f32, kind="ExternalOutput")

    with tile.TileContext(nc) as tc:
        with tc.tile_pool(name="persist", bufs=1) as pp, \
             tc.tile_pool(name="work", bufs=2) as wp, \
             tc.tile_pool(name="hnew", bufs=3) as hp, \
             tc.tile_pool(name="psP", bufs=2, space="PSUM") as psP, \
             tc.tile_pool(name="ps1", bufs=1, space="PSUM") as ps1, \
             tc.tile_pool(name="dram", bufs=2, space="DRAM") as dp:

            # ---- persistent SBUF tiles ----
            xsa = pp.tile([128, T * SW], f32, tag="xsa", name="xsa")
            xsba = pp.tile([AR, T * SW], f32, tag="xsba", name="xsba")
            w_sb = {nm: pp.tile(sh, f32, tag=nm, name=nm) for nm, sh in wnames}
            maska = pp.tile([128, 512], f32, tag="maska", name="maska")
            maskb = pp.tile([CL, 512], f32, tag="maskb", name="maskb")
            valid = pp.tile([1, 128], f32, tag="valid", name="valid")
            validp = pp.tile([128, 1], f32, tag="validp", name="validp")
            ident = pp.tile([128, 128], f32, tag="ident", name="ident")
            imain = pp.tile([128, H], f32, tag="imain", name="imain")
            ileft = pp.tile([CL, H], f32, tag="ileft", name="ileft")
            ys0a = pp.tile([128, W0 * SW], f32, tag="ys0a", name="ys0a")
            ys0b = pp.tile([BR, W0 * SW], f32, tag="ys0b", name="ys0b")
            ys5 = pp.tile([AR, W0 * 128], f32, tag="ys5", name="ys5")
            h1a = pp.tile([128, SW], f32, tag="h1a", name="h1a")
            h1b = pp.tile([CL, SW], f32, tag="h1b", name="h1b")
            h51 = pp.tile([AR, 128], f32, tag="h51", name="h51")

            # ---- init: loads + zero state ----
            for nm, _ in wnames:
                nc.sync.dma_start(w_sb[nm][:], d_w[nm][:])
            nc.sync.dma_start(maska[:], d_maska[:])
            nc.sync.dma_start(maskb[:], d_maskb[:])
            nc.sync.dma_start(valid[:], d_valid[:])
            nc.sync.dma_start(validp[:], d_validp[:])
            nc.sync.dma_start(ident[:], d_ident[:])
            nc.sync.dma_start(imain[:], d_imain[:])
            nc.sync.dma_start(ileft[:], d_ileft[:])
            tchunk = 8
            for t0 in range(0, T, tchunk):
                c0, c1 = t0 * SW, min(T, t0 + tchunk) * SW
                nc.sync.dma_start(xsa[:, c0:c1], d_xsa[:, c0:c1])
                nc.sync.dma_start(xsba[:, c0:c1], d_xsba[:, c0:c1])
            nc.vector.memset(ys0a[:], 0.0)
            nc.vector.memset(ys0b[:], 0.0)
            nc.gpsimd.memset(ys5[:], 0.0)
            nc.gpsimd.memset(h1a[:], 0.0)
            nc.gpsimd.memset(h1b[:], 0.0)
            nc.gpsimd.memset(h51[:], 0.0)
            # ones row of ys0b: shifted valid indicator, per slot (DMA: no
            # partition-alignment constraint)
            for s in range(W0):
                nc.sync.dma_start(ys0b[BR - 1:BR, s * SW + 4: s * SW + 132],
                                  d_valid[0:1, :])

            def step(l, t):
                """One GRU step of layer l producing h_t (1-based t)."""
                if l == 0:
                    xa, xbase = xsa, (t - 1) * SW
                    xA, xAb = xsba, (t - 1) * SW
                    xB = None
                    ha, hbase = ys0a, ((t - 1) % W0) * SW
                    hb = ys0b
                    hA, hAb = ys5, ((t - 1) % W0) * 128
                    hBb = ((t - 1) % W0) * SW + 4
                else:
                    xa, xbase = ys0a, (t % W0) * SW
                    xA, xAb = ys5, (t % W0) * 128
                    xB, xBb = ys0b, (t % W0) * SW + 4
                    ha, hbase = h1a, 0
                    hb = h1b
                    hA, hAb = h51, 0
                    hBb = 4

                P = psP.tile([128, G3], f32, tag=f"P{l}", name=f"P{l}")
                Q = ps1.tile([128, 512], f32, tag=f"Q{l}", name=f"Q{l}")

                # x-side: full 450-wide gate pre-activations (+ bias row)
                for k in range(KW):
                    nc.tensor.matmul(
                        P[:, 0:G3], xa[:, xbase + k: xbase + k + 128],
                        w_sb[f"wim{l}"][:, k * G3:(k + 1) * G3],
                        start=(k == 0), stop=False)
                nc.tensor.matmul(P[:, 0:G3], xA[0:AR, xAb: xAb + 128],
                                 w_sb[f"wilA{l}"][0:AR, 0:G3],
                                 start=False, stop=False)
                if xB is not None:
                    nc.tensor.matmul(P[:, 0:G3], xB[0:BR, xBb: xBb + 128],
                                     w_sb["wilB1"][0:BR, 0:G3],
                                     start=False, stop=False)
                # h-side [r|z]
                for k in range(KW):
                    nc.tensor.matmul(
                        P[:, 0:2 * H], ha[:, hbase + k: hbase + k + 128],
                        w_sb[f"whm{l}"][:, k * G3: k * G3 + 2 * H],
                        start=False, stop=False)
                nc.tensor.matmul(P[:, 0:2 * H], hA[0:AR, hAb: hAb + 128],
                                 w_sb[f"whlA{l}"][0:AR, 0:2 * H],
                                 start=False, stop=False)
                nc.tensor.matmul(P[:, 0:2 * H], hb[0:CL, hBb: hBb + 128],
                                 w_sb[f"whlB{l}"][0:CL, 0:2 * H],
                                 start=False, stop=True)
                # h-side [n] + identity pass (h_{t-1} pos-major) into Q
                for k in range(KW):
                    nc.tensor.matmul(
                        Q[:, 0:H], ha[:, hbase + k: hbase + k + 128],
                        w_sb[f"whm{l}"][:, k * G3 + 2 * H:(k + 1) * G3],
                        start=(k == 0), stop=False)
                nc.tensor.matmul(Q[:, 0:H], hA[0:AR, hAb: hAb + 128],
                                 w_sb[f"whlA{l}"][0:AR, 2 * H:G3],
                                 start=False, stop=False)
                nc.tensor.matmul(Q[:, 0:H], hb[0:CL, hBb: hBb + 128],
                                 w_sb[f"whlB{l}"][0:CL, 2 * H:G3],
                                 start=False, stop=False)
                nc.tensor.matmul(Q[:, 256:256 + H],
                                 ha[:, hbase + 2: hbase + 2 + 128],
                                 imain[:], start=False, stop=False)
                nc.tensor.matmul(Q[:, 256:256 + H],
                                 hb[0:CL, hbase + 2: hbase + 2 + 128],
                                 ileft[:], start=False, stop=True)

                # gates
                import concourse.mybir as mybir
                rz = wp.tile([128, 2 * H], f32, tag=f"rz{l}", name=f"rz{l}")
                nc.scalar.activation(rz[:], P[:, 0:2 * H],
                                     mybir.ActivationFunctionType.Sigmoid)
                ta = wp.tile([128, H], f32, tag=f"ta{l}", name=f"ta{l}")
                nc.vector.tensor_mul(ta[:], Q[:, 0:H], rz[:, 0:H])
                tb = wp.tile([128, H], f32, tag=f"tb{l}", name=f"tb{l}")
                nc.vector.tensor_add(tb[:], ta[:], P[:, 2 * H:G3])
                tn = wp.tile([128, H], f32, tag=f"tn{l}", name=f"tn{l}")
                # scale by the valid-position indicator: pins h to exactly 0
                # outside the global [0,L) range (reference SAME padding)
                nc.scalar.activation(tn[:], tb[:],
                                     mybir.ActivationFunctionType.Tanh,
                                     scale=validp[:, 0:1])
                tc_ = wp.tile([128, H], f32, tag=f"tc{l}", name=f"tc{l}")
                nc.vector.tensor_sub(tc_[:], Q[:, 256:256 + H], tn[:])
                td = wp.tile([128, H], f32, tag=f"td{l}", name=f"td{l}")
                nc.vector.tensor_mul(td[:], rz[:, H:2 * H], tc_[:])
                hn = hp.tile([128, H], f32, tag=f"hn{l}", name=f"hn{l}")
                nc.vector.tensor_add(hn[:], tn[:], td[:])

                # transpose h_new back to channel-major
                TT = ps1.tile([128, 256], f32, tag=f"T{l}", name=f"T{l}")
                nc.tensor.matmul(TT[:, 0:128], hn[:, 0:128], ident[:],
                                 is_transpose=True, start=True, stop=False)
                nc.tensor.matmul(TT[0:CL, 128:256], hn[:, 128:H], ident[:],
                                 is_transpose=True, start=False, stop=True)
                if l == 0:
                    st = (t % W0) * SW
                    nc.vector.tensor_copy(ys0a[:, st + 2: st + 130],
                                          TT[:, 0:128])
                    nc.vector.tensor_copy(ys0b[0:CL, st + 2: st + 130],
                                          TT[0:CL, 128:256])
                else:
                    nc.vector.tensor_copy(h1a[:, 2:130], TT[:, 0:128])
                    nc.vector.tensor_copy(h1b[0:CL, 2:130], TT[0:CL, 128:256])
                    nc.sync.dma_start(d_out[t - 1, :, :], hn[32:96, 0:H])
                return hn

            def replicas(l, t):
                """Rebuild 32-aligned tap blocks 0..3 (post-patch if any)."""
                if l == 0:
                    st = (t % W0) * SW
                    s5 = (t % W0) * 128
                    for k in range(4):
                        nc.vector.tensor_copy(
                            ys5[k * 32: k * 32 + CL, s5: s5 + 128],
                            ys0b[0:CL, st + k: st + k + 128])
                else:
                    for k in range(4):
                        nc.vector.tensor_copy(
                            h51[k * 32: k * 32 + CL, 0:128],
                            h1b[0:CL, k: k + 128])

            def exchange(l, t):
                """ReduceScatter halo refresh of layer l's state at step t."""
                import concourse.mybir as mybir
                if l == 0:
                    A, B, base = ys0a, ys0b, (t % W0) * SW
                else:
                    A, B, base = h1a, h1b, 0
                rsa = wp.tile([128, 512], f32, tag="rsa", name="rsa")
                rsb = wp.tile([CL, 512], f32, tag="rsb", name="rsb")
                # q in [0,32): receivers' left halo <- my owned last 32
                # q in [32,64): receivers' right halo <- my owned first 32
                for dq, sc in ((0, 66), (32, 34)):
                    nc.vector.tensor_mul(
                        rsa[:].rearrange("c (s q) -> c s q", s=8)[:, :, dq:dq + 32],
                        A[:, base + sc: base + sc + 32]
                        .unsqueeze(1).broadcast_to([128, 8, 32]),
                        maska[:].rearrange("c (s q) -> c s q", s=8)[:, :, dq:dq + 32])
                    nc.vector.tensor_mul(
                        rsb[0:CL].rearrange("c (s q) -> c s q", s=8)[:, :, dq:dq + 32],
                        B[0:CL, base + sc: base + sc + 32]
                        .unsqueeze(1).broadcast_to([CL, 8, 32]),
                        maskb[:].rearrange("c (s q) -> c s q", s=8)[:, :, dq:dq + 32])
                rs_in = dp.tile([8, C, 64], f32, tag="rs_in", name="rs_in")
                rs_out = dp.tile([C, 64], f32, tag="rs_out", name="rs_out")
                nc.sync.dma_start(rs_in[:, 0:128, :].transpose([1, 0, 2]),
                                  rsa[:].rearrange("c (s q) -> c s q", s=8))
                nc.sync.dma_start(rs_in[:, 128:C, :].transpose([1, 0, 2]),
                                  rsb[0:CL].rearrange("c (s q) -> c s q", s=8))
                nc.gpsimd.collective_compute(
                    "ReduceScatter", mybir.AluOpType.add,
                    replica_groups=[list(range(NCORES))],
                    ins=[rs_in[:].opt()], outs=[rs_out[:].opt()])
                nc.sync.dma_start(A[:, base + 2: base + 34], rs_out[0:128, 0:32])
                nc.sync.dma_start(A[:, base + 98: base + 130],
                                  rs_out[0:128, 32:64])
                nc.sync.dma_start(B[0:CL, base + 2: base + 34],
                                  rs_out[128:C, 0:32])
                nc.sync.dma_start(B[0:CL, base + 98: base + 130],
                                  rs_out[128:C, 32:64])

            # ---- main pipelined emission ----
            for tt in range(1, T + LAG + 1):
                if tt <= T:
                    step(0, tt)
                    if tt % 16 == 0:
                        exchange(0, tt)
                    replicas(0, tt)
                s = tt - LAG
                if 1 <= s <= T:
                    step(1, s)
                    if s % 16 == 8 and s < T:
                        exchange(1, s)
                    replicas(1, s)

    nc.compile()
    return nc


def prep_inputs(xs, W_i0, b_i0, W_h0, W_i1, b_i1, W_h1, T):
    """Host-side sharding/packing -> per-core in_maps."""
    xs = np.asarray(xs, np.float32)
    pads = 34
    xs_p = np.zeros((L + 2 * pads + 4, T, C), np.float32)
    xs_p[pads:pads + L] = xs[:, :T]

    def pack_w(Wi, bi, Wh):
        Wi = np.asarray(Wi, np.float32)
        Wh = np.asarray(Wh, np.float32)
        wim = np.ascontiguousarray(Wi.transpose(1, 0, 2)[:128]).reshape(128, KW * G3)
        whm = np.ascontiguousarray(Wh.transpose(1, 0, 2)[:128]).reshape(128, KW * G3)
        wilA = np.zeros((AR, G3), np.float32)
        whlA = np.zeros((AR, G3), np.float32)
        for k in range(4):
            wilA[k * 32: k * 32 + CL] = Wi[k, 128:C, :]
            whlA[k * 32: k * 32 + CL] = Wh[k, 128:C, :]
        # layer-0 only: tap-4 + bias packed into the gap rows
        wil0 = wilA.copy()
        for r, ch in _GAP_PACK:
            if ch < CL:
                wil0[r] = Wi[4, 128 + ch, :]
        wil0[_ONES_ROW] = np.asarray(bi, np.float32)
        wilB = np.zeros((BR, G3), np.float32)
        wilB[:CL] = Wi[4, 128:C, :]
        wilB[CL] = np.asarray(bi, np.float32)
        whlB = np.ascontiguousarray(Wh[4, 128:C, :])
        return wim, whm, wilA, wil0, wilB, whlA, whlB

    packed = [pack_w(W_i0, b_i0, W_h0), pack_w(W_i1, b_i1, W_h1)]
    ident = np.eye(128, dtype=np.float32)
    imain = np.eye(128, H, dtype=np.float32)
    ileft = np.zeros((CL, H), np.float32)
    for j in range(CL):
        ileft[j, 128 + j] = 1.0

    in_maps = []
    for i in range(NCORES):
        blk = xs_p[OWN * i: OWN * i + SW]          # (134, T, C)
        blkT = np.ascontiguousarray(blk.transpose(2, 1, 0))  # (C, T, 134)
        xsa = blkT[:128].reshape(128, T * SW)
        xsba = np.zeros((AR, T, SW), np.float32)
        for k in range(4):
            xsba[k * 32: k * 32 + CL, :, 0:128] = \
                blk[k:k + 128, :, 128:C].transpose(2, 1, 0)
        pos = np.arange(128) + OWN * i - 32
        validv = ((pos >= 0) & (pos < L)).astype(np.float32)
        tap4 = blk[4:4 + 128, :, 128:C].transpose(2, 1, 0)  # (CL, T, 128)
        for r, ch in _GAP_PACK:
            if ch < CL:
                xsba[r, :, 0:128] = tap4[ch]
        xsba[_ONES_ROW, :, 0:128] = validv[None, :]

        mask = np.zeros((8, 64), np.float32)
        if i + 1 < NCORES:
            mask[i + 1, 0:32] = 1.0
        if i - 1 >= 0:
            mask[i - 1, 32:64] = 1.0
        maska = np.tile(mask.reshape(1, 512), (128, 1))

        im = {
            "xsT_a": xsa,
            "xsT_bA": np.ascontiguousarray(xsba.reshape(AR, T * SW)),
            "mask_a": np.ascontiguousarray(maska),
            "mask_b": np.ascontiguousarray(maska[:CL]),
            "valid128": validv.reshape(1, 128),
            "validp": np.ascontiguousarray(validv.reshape(128, 1)),
            "ident": ident, "imain": imain, "ileft": ileft,
        }
        for l in range(2):
            wim, whm, wilA, wil0, wilB, whlA, whlB = packed[l]
            im[f"wim{l}"] = wim
            im[f"whm{l}"] = whm
            im[f"wilA{l}"] = wil0 if l == 0 else wilA
            im[f"whlA{l}"] = whlA
            im[f"whlB{l}"] = whlB
        im["wilB1"] = packed[1][4]
        in_maps.append(im)
    return in_maps


_BUILD_CACHE = {}


def run(inputs, T=96, trace=False):
    global LAST_EXEC_NS
    from concourse import bass_utils
    if T not in _BUILD_CACHE:
        _BUILD_CACHE[T] = build(T)
    nc = _BUILD_CACHE[T]
    in_maps = prep_inputs(T=T, **inputs)
    if trace:
        _install_ntff_hook()
    res = bass_utils.run_bass_kernel_spmd(
        nc, in_maps, core_ids=list(range(NCORES)), trace=trace)
    LAST_EXEC_NS = res.exec_time_ns
    ys = np.empty((L, T, H), np.float32)
    for i in range(NCORES):
        ys[OWN * i: OWN * (i + 1)] = res.results[i]["out"].transpose(1, 0, 2)
    return ys


def kernel(**inputs):
    trace = bool(int(os.environ.get("BASS_KERNEL_TRACE", "0")))
    return run(inputs, T=96, trace=trace)


# revision 7
# speedup vs baseline: 2.7326x; 2.7326x over previous
"""Trainium2 Bass kernel for a 2-layer ConvGRU (L=512, T=96, C=H=150, K=5).

Sharding: spatial axis L split over 8 NeuronCores (64 owned positions each).
Each core computes a 128-wide region (owned + 32 halo per side). Halo
validity decays 2 positions/step (kernel-5 SAME conv); it is refreshed every
16 steps by a ReduceScatter halo exchange (per-core one-hot masks route each
core's boundary strips to its neighbours' slots, keeping the SPMD program
uniform). Layer-0 exchanges sit at t=16k, layer-1 at t=16k+8, so each
layer's collective latency hides under the other layer's compute.

TensorEngine work per GRU step and layer: 5 main-channel taps + a packed
4-tap channel-remainder block (32-aligned sub-blocks, zero gap rows) + a
tap-4/bias block, per gate group ([r|z] into one PSUM bank, [n] into
another), plus a 2-matmul identity pass that re-materialises h_{t-1}
position-major straight from the exchanged channel-major state. Gate math
runs on ACT (sigmoid/tanh) + DVE; h_new returns to channel-major via two PE
transposes.

All input reshaping (channel-major transposes, tap-shifted im2col packing,
bias/valid rows, routing masks) is done host-side in numpy inside kernel().
"""

import os
import sys
import types

import numpy as np
import ml_dtypes

BF16 = ml_dtypes.bfloat16

if "/opt/trn_rl_repo" not in sys.path:
    sys.path.insert(0, "/opt/trn_rl_repo")


def _install_ntff_hook():
    # antenv.axon_hooks is absent from this image; recreate the registry and
    # register the ctypes NTFF hook so trace=True yields exec_time_ns.
    try:
        import antenv
        if "antenv.axon_hooks" in sys.modules:
            return True
        mod = types.ModuleType("antenv.axon_hooks")
        _hook = [None]
        mod.set_axon_ntff_profile_hook = lambda h: _hook.__setitem__(0, h)
        mod.get_axon_ntff_profile_hook = lambda: _hook[0]
        sys.modules["antenv.axon_hooks"] = mod
        antenv.axon_hooks = mod
        from trn_agent_boot.trn_boot import _ntff_profile_via_ctypes
        mod.set_axon_ntff_profile_hook(
            _ntff_profile_via_ctypes("/opt/axon/libaxon_pjrt.so"))
        return True
    except Exception:
        return False


L, C, H, KW = 512, 150, 150, 5
G3 = 3 * H            # 450
NCORES = 8
OWN = 64              # owned positions per core
SW = 134              # stored width per timestep (128 region + 2 pads/side)
W0 = 12               # rolling slots for layer-0 output trajectory
LAG = 4               # emission lag of layer 1 behind layer 0
CL = 22               # channel remainder (150 - 128)
AR = 118              # 4 tap blocks at partitions {0,32,64,96}, 22 rows each
BR = 23               # tap-4 rows + ones/bias row

# tap-4 channel -> spare row inside the 118-row aligned block (layer-0 xs
# packing only): gaps 22..31, 54..63, 86..87; ones/bias row at 88.
_GAP_PACK = [(22 + c, c) for c in range(10)] + \
    [(54 + c - 10, c) for c in range(10, 20)] + \
    [(86 + c - 20, c) for c in range(20, 22)]
_GAP_PACK = [(r, c) for r, c in _GAP_PACK]
_ONES_ROW = 88

LAST_EXEC_NS = None


def build(T):
    import concourse.bacc as bacc
    import concourse.mybir as mybir
    import concourse.tile as tile

    f32 = mybir.dt.float32
    bf16 = mybir.dt.bfloat16
    nc = bacc.Bacc("TRN2", target_bir_lowering=False, debug=False,
                   num_devices=NCORES)

    d_xsa = nc.dram_tensor("xsT_a", [128, T * SW], bf16, kind="ExternalInput")
    d_xsba = nc.dram_tensor("xsT_bA", [AR, T * SW], bf16, kind="ExternalInput")
    wnames = []
    for l in range(2):
        for nm in (f"wim{l}", f"whm{l}"):
            wnames.append((nm, [128, KW * G3]))
        wnames.append((f"wilA{l}", [AR, G3]))
        wnames.append((f"whlA{l}", [AR, G3]))
        wnames.append((f"whlB{l}", [CL, G3]))
    wnames.append(("wilB1", [BR, G3]))
    d_w = {nm: nc.dram_tensor(nm, sh, bf16, kind="ExternalInput")
           for nm, sh in wnames}
    d_maska = nc.dram_tensor("mask_a", [128, 512], bf16, kind="ExternalInput")
    d_maskb = nc.dram_tensor("mask_b", [CL, 512], bf16, kind="ExternalInput")
    d_valid = nc.dram_tensor("valid128", [1, 128], bf16, kind="ExternalInput")
    d_validp = nc.dram_tensor("validp", [128, 1], f32, kind="ExternalInput")
    d_ident = nc.dram_tensor("ident", [128, 128], f32, kind="ExternalInput")
    d_imain = nc.dram_tensor("imain", [128, H], bf16, kind="ExternalInput")
    d_ileft = nc.dram_tensor("ileft", [CL, H], bf16, kind="ExternalInput")
    d_out = nc.dram_tensor("out", [T, OWN, H], f32, kind="ExternalOutput")

    with tile.TileContext(nc) as tc:
        with tc.tile_pool(name="persist", bufs=1) as pp, \
             tc.tile_pool(name="work", bufs=2) as wp, \
             tc.tile_pool(name="hnew", bufs=3) as hp, \
             tc.tile_pool(name="psP", bufs=2, space="PSUM") as psP, \
             tc.tile_pool(name="ps1", bufs=1, space="PSUM") as ps1, \
             tc.tile_pool(name="dram", bufs=2, space="DRAM") as dp:

            # ---- persistent SBUF tiles ----
            xsa = pp.tile([128, T * SW], bf16, tag="xsa", name="xsa")
            xsba = pp.tile([AR, T * SW], bf16, tag="xsba", name="xsba")
            w_sb = {nm: pp.tile(sh, bf16, tag=nm, name=nm) for nm, sh in wnames}
            maska = pp.tile([128, 512], bf16, tag="maska", name="maska")
            maskb = pp.tile([CL, 512], bf16, tag="maskb", name="maskb")
            valid = pp.tile([1, 128], bf16, tag="valid", name="valid")
            validp = pp.tile([128, 1], f32, tag="validp", name="validp")
            ident = pp.tile([128, 128], f32, tag="ident", name="ident")
            imain = pp.tile([128, H], bf16, tag="imain", name="imain")
            ileft = pp.tile([CL, H], bf16, tag="ileft", name="ileft")
            ys0a = pp.tile([128, W0 * SW], bf16, tag="ys0a", name="ys0a")
            ys0b = pp.tile([BR, W0 * SW], bf16, tag="ys0b", name="ys0b")
            ys5 = pp.tile([AR, W0 * 128], bf16, tag="ys5", name="ys5")
            h1a = pp.tile([128, SW], bf16, tag="h1a", name="h1a")
            h1b = pp.tile([CL, SW], bf16, tag="h1b", name="h1b")
            h51 = pp.tile([AR, 128], bf16, tag="h51", name="h51")

            # ---- init: loads + zero state ----
            for nm, _ in wnames:
                nc.sync.dma_start(w_sb[nm][:], d_w[nm][:])
            nc.sync.dma_start(maska[:], d_maska[:])
            nc.sync.dma_start(maskb[:], d_maskb[:])
            nc.sync.dma_start(valid[:], d_valid[:])
            nc.sync.dma_start(validp[:], d_validp[:])
            nc.sync.dma_start(ident[:], d_ident[:])
            nc.sync.dma_start(imain[:], d_imain[:])
            nc.sync.dma_start(ileft[:], d_ileft[:])
            tchunk = 8
            for t0 in range(0, T, tchunk):
                c0, c1 = t0 * SW, min(T, t0 + tchunk) * SW
                nc.sync.dma_start(xsa[:, c0:c1], d_xsa[:, c0:c1])
                nc.sync.dma_start(xsba[:, c0:c1], d_xsba[:, c0:c1])
            nc.vector.memset(ys0a[:], 0.0)
            nc.vector.memset(ys0b[:], 0.0)
            nc.gpsimd.memset(ys5[:], 0.0)
            nc.gpsimd.memset(h1a[:], 0.0)
            nc.gpsimd.memset(h1b[:], 0.0)
            nc.gpsimd.memset(h51[:], 0.0)
            # ones row of ys0b: shifted valid indicator, per slot (DMA: no
            # partition-alignment constraint)
            for s in range(W0):
                nc.sync.dma_start(ys0b[BR - 1:BR, s * SW + 4: s * SW + 132],
                                  d_valid[0:1, :])

            def step(l, t):
                """One GRU step of layer l producing h_t (1-based t)."""
                if l == 0:
                    xa, xbase = xsa, (t - 1) * SW
                    xA, xAb = xsba, (t - 1) * SW
                    xB = None
                    ha, hbase = ys0a, ((t - 1) % W0) * SW
                    hb = ys0b
                    hA, hAb = ys5, ((t - 1) % W0) * 128
                    hBb = ((t - 1) % W0) * SW + 4
                else:
                    xa, xbase = ys0a, (t % W0) * SW
                    xA, xAb = ys5, (t % W0) * 128
                    xB, xBb = ys0b, (t % W0) * SW + 4
                    ha, hbase = h1a, 0
                    hb = h1b
                    hA, hAb = h51, 0
                    hBb = 4

                P = psP.tile([128, G3], f32, tag=f"P{l}", name=f"P{l}")
                Q = ps1.tile([128, 512], f32, tag=f"Q{l}", name=f"Q{l}")

                # x-side: full 450-wide gate pre-activations (+ bias row)
                for k in range(KW):
                    nc.tensor.matmul(
                        P[:, 0:G3], xa[:, xbase + k: xbase + k + 128],
                        w_sb[f"wim{l}"][:, k * G3:(k + 1) * G3],
                        start=(k == 0), stop=False)
                nc.tensor.matmul(P[:, 0:G3], xA[0:AR, xAb: xAb + 128],
                                 w_sb[f"wilA{l}"][0:AR, 0:G3],
                                 start=False, stop=False)
                if xB is not None:
                    nc.tensor.matmul(P[:, 0:G3], xB[0:BR, xBb: xBb + 128],
                                     w_sb["wilB1"][0:BR, 0:G3],
                                     start=False, stop=False)
                # h-side [r|z]
                for k in range(KW):
                    nc.tensor.matmul(
                        P[:, 0:2 * H], ha[:, hbase + k: hbase + k + 128],
                        w_sb[f"whm{l}"][:, k * G3: k * G3 + 2 * H],
                        start=False, stop=False)
                nc.tensor.matmul(P[:, 0:2 * H], hA[0:AR, hAb: hAb + 128],
                                 w_sb[f"whlA{l}"][0:AR, 0:2 * H],
                                 start=False, stop=False)
                nc.tensor.matmul(P[:, 0:2 * H], hb[0:CL, hBb: hBb + 128],
                                 w_sb[f"whlB{l}"][0:CL, 0:2 * H],
                                 start=False, stop=True)
                # h-side [n] + identity pass (h_{t-1} pos-major) into Q
                for k in range(KW):
                    nc.tensor.matmul(
                        Q[:, 0:H], ha[:, hbase + k: hbase + k + 128],
                        w_sb[f"whm{l}"][:, k * G3 + 2 * H:(k + 1) * G3],
                        start=(k == 0), stop=False)
                nc.tensor.matmul(Q[:, 0:H], hA[0:AR, hAb: hAb + 128],
                                 w_sb[f"whlA{l}"][0:AR, 2 * H:G3],
                                 start=False, stop=False)
                nc.tensor.matmul(Q[:, 0:H], hb[0:CL, hBb: hBb + 128],
                                 w_sb[f"whlB{l}"][0:CL, 2 * H:G3],
                                 start=False, stop=False)
                nc.tensor.matmul(Q[:, 256:256 + H],
                                 ha[:, hbase + 2: hbase + 2 + 128],
                                 imain[:], start=False, stop=False)
                nc.tensor.matmul(Q[:, 256:256 + H],
                                 hb[0:CL, hbase + 2: hbase + 2 + 128],
                                 ileft[:], start=False, stop=True)

                # gates
                import concourse.mybir as mybir
                rz = wp.tile([128, 2 * H], f32, tag=f"rz{l}", name=f"rz{l}")
                nc.scalar.activation(rz[:], P[:, 0:2 * H],
                                     mybir.ActivationFunctionType.Sigmoid)
                ta = wp.tile([128, H], f32, tag=f"ta{l}", name=f"ta{l}")
                nc.vector.tensor_mul(ta[:], Q[:, 0:H], rz[:, 0:H])
                tb = wp.tile([128, H], f32, tag=f"tb{l}", name=f"tb{l}")
                nc.vector.tensor_add(tb[:], ta[:], P[:, 2 * H:G3])
                tn = wp.tile([128, H], f32, tag=f"tn{l}", name=f"tn{l}")
                # scale by the valid-position indicator: pins h to exactly 0
                # outside the global [0,L) range (reference SAME padding)
                nc.scalar.activation(tn[:], tb[:],
                                     mybir.ActivationFunctionType.Tanh,
                                     scale=validp[:, 0:1])
                tc_ = wp.tile([128, H], f32, tag=f"tc{l}", name=f"tc{l}")
                nc.vector.tensor_sub(tc_[:], Q[:, 256:256 + H], tn[:])
                td = wp.tile([128, H], f32, tag=f"td{l}", name=f"td{l}")
                nc.vector.tensor_mul(td[:], rz[:, H:2 * H], tc_[:])
                hn = hp.tile([128, H], f32, tag=f"hn{l}", name=f"hn{l}")
                nc.vector.tensor_add(hn[:], tn[:], td[:])

                # transpose h_new back to channel-major
                TT = ps1.tile([128, 256], f32, tag=f"T{l}", name=f"T{l}")
                nc.tensor.matmul(TT[:, 0:128], hn[:, 0:128], ident[:],
                                 is_transpose=True, start=True, stop=False)
                nc.tensor.matmul(TT[0:CL, 128:256], hn[:, 128:H], ident[:],
                                 is_transpose=True, start=False, stop=True)
                if l == 0:
                    st = (t % W0) * SW
                    nc.vector.tensor_copy(ys0a[:, st + 2: st + 130],
                                          TT[:, 0:128])
                    nc.vector.tensor_copy(ys0b[0:CL, st + 2: st + 130],
                                          TT[0:CL, 128:256])
                else:
                    nc.vector.tensor_copy(h1a[:, 2:130], TT[:, 0:128])
                    nc.vector.tensor_copy(h1b[0:CL, 2:130], TT[0:CL, 128:256])
                    nc.sync.dma_start(d_out[t - 1, :, :], hn[32:96, 0:H])
                return hn

            def replicas(l, t):
                """Rebuild 32-aligned tap blocks 0..3 (post-patch if any)."""
                if l == 0:
                    st = (t % W0) * SW
                    s5 = (t % W0) * 128
                    for k in range(4):
                        nc.vector.tensor_copy(
                            ys5[k * 32: k * 32 + CL, s5: s5 + 128],
                            ys0b[0:CL, st + k: st + k + 128])
                else:
                    for k in range(4):
                        nc.vector.tensor_copy(
                            h51[k * 32: k * 32 + CL, 0:128],
                            h1b[0:CL, k: k + 128])

            def exchange(l, t):
                """ReduceScatter halo refresh of layer l's state at step t."""
                import concourse.mybir as mybir
                if l == 0:
                    A, B, base = ys0a, ys0b, (t % W0) * SW
                else:
                    A, B, base = h1a, h1b, 0
                rsa = wp.tile([128, 512], bf16, tag="rsa", name="rsa")
                rsb = wp.tile([CL, 512], bf16, tag="rsb", name="rsb")
                # q in [0,32): receivers' left halo <- my owned last 32
                # q in [32,64): receivers' right halo <- my owned first 32
                for dq, sc in ((0, 66), (32, 34)):
                    nc.vector.tensor_mul(
                        rsa[:].rearrange("c (s q) -> c s q", s=8)[:, :, dq:dq + 32],
                        A[:, base + sc: base + sc + 32]
                        .unsqueeze(1).broadcast_to([128, 8, 32]),
                        maska[:].rearrange("c (s q) -> c s q", s=8)[:, :, dq:dq + 32])
                    nc.vector.tensor_mul(
                        rsb[0:CL].rearrange("c (s q) -> c s q", s=8)[:, :, dq:dq + 32],
                        B[0:CL, base + sc: base + sc + 32]
                        .unsqueeze(1).broadcast_to([CL, 8, 32]),
                        maskb[:].rearrange("c (s q) -> c s q", s=8)[:, :, dq:dq + 32])
                rs_in = dp.tile([8, C, 64], bf16, tag="rs_in", name="rs_in")
                rs_out = dp.tile([C, 64], bf16, tag="rs_out", name="rs_out")
                nc.sync.dma_start(rs_in[:, 0:128, :].transpose([1, 0, 2]),
                                  rsa[:].rearrange("c (s q) -> c s q", s=8))
                nc.sync.dma_start(rs_in[:, 128:C, :].transpose([1, 0, 2]),
                                  rsb[0:CL].rearrange("c (s q) -> c s q", s=8))
                nc.gpsimd.collective_compute(
                    "ReduceScatter", mybir.AluOpType.add,
                    replica_groups=[list(range(NCORES))],
                    ins=[rs_in[:].opt()], outs=[rs_out[:].opt()])
                nc.sync.dma_start(A[:, base + 2: base + 34], rs_out[0:128, 0:32])
                nc.sync.dma_start(A[:, base + 98: base + 130],
                                  rs_out[0:128, 32:64])
                nc.sync.dma_start(B[0:CL, base + 2: base + 34],
                                  rs_out[128:C, 0:32])
                nc.sync.dma_start(B[0:CL, base + 98: base + 130],
                                  rs_out[128:C, 32:64])

            # ---- main pipelined emission ----
            for tt in range(1, T + LAG + 1):
                if tt <= T:
                    step(0, tt)
                    if tt % 16 == 0:
                        exchange(0, tt)
                    replicas(0, tt)
                s = tt - LAG
                if 1 <= s <= T:
                    step(1, s)
                    if s % 16 == 8 and s < T:
                        exchange(1, s)
                    replicas(1, s)

    nc.compile()
    return nc


def prep_inputs(xs, W_i0, b_i0, W_h0, W_i1, b_i1, W_h1, T):
    """Host-side sharding/packing -> per-core in_maps."""
    xs = np.asarray(xs, np.float32)
    pads = 34
    xs_p = np.zeros((L + 2 * pads + 4, T, C), np.float32)
    xs_p[pads:pads + L] = xs[:, :T]

    def pack_w(Wi, bi, Wh):
        Wi = np.asarray(Wi, np.float32)
        Wh = np.asarray(Wh, np.float32)
        wim = np.ascontiguousarray(Wi.transpose(1, 0, 2)[:128]).reshape(128, KW * G3)
        whm = np.ascontiguousarray(Wh.transpose(1, 0, 2)[:128]).reshape(128, KW * G3)
        wilA = np.zeros((AR, G3), np.float32)
        whlA = np.zeros((AR, G3), np.float32)
        for k in range(4):
            wilA[k * 32: k * 32 + CL] = Wi[k, 128:C, :]
            whlA[k * 32: k * 32 + CL] = Wh[k, 128:C, :]
        # layer-0 only: tap-4 + bias packed into the gap rows
        wil0 = wilA.copy()
        for r, ch in _GAP_PACK:
            if ch < CL:
                wil0[r] = Wi[4, 128 + ch, :]
        wil0[_ONES_ROW] = np.asarray(bi, np.float32)
        wilB = np.zeros((BR, G3), np.float32)
        wilB[:CL] = Wi[4, 128:C, :]
        wilB[CL] = np.asarray(bi, np.float32)
        whlB = np.ascontiguousarray(Wh[4, 128:C, :])
        cv = lambda a: a.astype(BF16)
        return (cv(wim), cv(whm), cv(wilA), cv(wil0), cv(wilB), cv(whlA),
                cv(whlB))

    packed = [pack_w(W_i0, b_i0, W_h0), pack_w(W_i1, b_i1, W_h1)]
    ident = np.eye(128, dtype=np.float32)
    imain = np.eye(128, H, dtype=np.float32)
    ileft = np.zeros((CL, H), np.float32)
    for j in range(CL):
        ileft[j, 128 + j] = 1.0

    in_maps = []
    for i in range(NCORES):
        blk = xs_p[OWN * i: OWN * i + SW]          # (134, T, C)
        blkT = np.ascontiguousarray(blk.transpose(2, 1, 0))  # (C, T, 134)
        xsa = blkT[:128].reshape(128, T * SW)
        xsba = np.zeros((AR, T, SW), np.float32)
        for k in range(4):
            xsba[k * 32: k * 32 + CL, :, 0:128] = \
                blk[k:k + 128, :, 128:C].transpose(2, 1, 0)
        pos = np.arange(128) + OWN * i - 32
        validv = ((pos >= 0) & (pos < L)).astype(np.float32)
        tap4 = blk[4:4 + 128, :, 128:C].transpose(2, 1, 0)  # (CL, T, 128)
        for r, ch in _GAP_PACK:
            if ch < CL:
                xsba[r, :, 0:128] = tap4[ch]
        xsba[_ONES_ROW, :, 0:128] = validv[None, :]

        mask = np.zeros((8, 64), np.float32)
        if i + 1 < NCORES:
            mask[i + 1, 0:32] = 1.0
        if i - 1 >= 0:
            mask[i - 1, 32:64] = 1.0
        maska = np.tile(mask.reshape(1, 512), (128, 1))

        im = {
            "xsT_a": xsa.astype(BF16),
            "xsT_bA": np.ascontiguousarray(xsba.reshape(AR, T * SW)).astype(BF16),
            "mask_a": np.ascontiguousarray(maska).astype(BF16),
            "mask_b": np.ascontiguousarray(maska[:CL]).astype(BF16),
            "valid128": validv.reshape(1, 128).astype(BF16),
            "validp": np.ascontiguousarray(validv.reshape(128, 1)),
            "ident": ident, "imain": imain.astype(BF16),
            "ileft": ileft.astype(BF16),
        }
        for l in range(2):
            wim, whm, wilA, wil0, wilB, whlA, whlB = packed[l]
            im[f"wim{l}"] = wim
            im[f"whm{l}"] = whm
            im[f"wilA{l}"] = wil0 if l == 0 else wilA
            im[f"whlA{l}"] = whlA
            im[f"whlB{l}"] = whlB
        im["wilB1"] = packed[1][4]
        in_maps.append(im)
    return in_maps


_BUILD_CACHE = {}


def run(inputs, T=96, trace=False):
    global LAST_EXEC_NS
    from concourse import bass_utils
    if T not in _BUILD_CACHE:
        _BUILD_CACHE[T] = build(T)
    nc = _BUILD_CACHE[T]
    in_maps = prep_inputs(T=T, **inputs)
    if trace:
        _install_ntff_hook()
    res = bass_utils.run_bass_kernel_spmd(
        nc, in_maps, core_ids=list(range(NCORES)), trace=trace)
    LAST_EXEC_NS = res.exec_time_ns
    ys = np.empty((L, T, H), np.float32)
    for i in range(NCORES):
        ys[OWN * i: OWN * (i + 1)] = res.results[i]["out"].transpose(1, 0, 2)
    return ys


def kernel(**inputs):
    trace = bool(int(os.environ.get("BASS_KERNEL_TRACE", "0")))
    return run(inputs, T=96, trace=trace)
